# revision 11
# baseline (speedup 1.0000x reference)
"""Contrastive patch loss (InfoNCE over sampled voxel patches) on 8 TRN2 NeuronCores.

Math
----
Reference computes, per patch p and batch b, cs[k,l] = <t2n[:,i_pk], t1n[:,i_pl]>
over k=512 sampled voxels (i = idx[p]), e = exp(cs/bw), then the mean over
(p,b,j) of -log(0.5*e_jj*(1/colsum_j + 1/rowsum_j) + eps).

Since every sampled voxel index lives in [0, 512), cs is a gather of the
512x512 Gram matrix G_b = t2n^T @ t1n.  With E_b = exp(G_b/bw) and c_p[s] the
multiplicity of voxel s in patch p:

    colsum_j = (E_b^T @ c_p)[i_j]      rowsum_j = (E_b @ c_p)[i_j]
    pos_j    = diag(E_b)[i_j]

    loss = -1/(P*B*K) * sum_{b,p,s} c_p[s] *
           log(0.5*diagE_b[s]*(1/CS_b[s,p] + 1/RS_b[s,p]) + eps)

Sharding: 8 cores = 2 batches x 4 column-blocks of E.  Core (b,q) permutes
the voxel order so its block q comes first, then computes ONLY the 128-column
blocks it needs:

    E  col-block:  E[a-blk, 0-blk]  = exp(s2[a] (x) s1[0] * fx_a^T fyn)   (CS)
    E^T col-block: E^T[a-blk,0-blk] = exp(s1[a] (x) s2[0] * fy_a^T fxn)   (RS)

where fyn/fxn are the 128 moving columns pre-normalized (column scale folded
into the fp8 operand) and the per-partition scales s*[a] are applied by the
exp ACT.  CS[0-blk] / RS[0-blk] are then plain 4-term PSUM-accumulated
matmuls against the count columns -- no gathers, and no core ever
materializes the full E.  The (0,0) E^T tile is the (0,0) E tile PE-transposed
(one fewer Gram + exp).  Per-core partial sums return as a (128,4) tile and
are summed on the host (no collectives).

Engine split: DVE squares the q-block features (one 4-D strided TT) and
normalizes; ACT squares the fx rest-blocks (idle window during input DMA) and
runs all ln/exp; GpSimd squares the fy rest-blocks; PE does channel sums,
Grams, CS/RS.  E/E^T tiles live in per-block SBUF tiles so the exp stream is
not false-serialized against the CS/RS reads (tile-granular WAR deps).

Precision: features fp8e4m3 (DoubleRow matmuls, 256-deep at half cycles/row);
norms from squares of the same fp8 values; E/E^T fp8 with a constant exponent
offset (cancels in pos/CS, pos/RS); accumulation, exp/log, loss fp32.
"""

import math
import os

import ml_dtypes
import numpy as np

import concourse.bacc as bacc
import concourse.tile as tile
from concourse import hw_specs, mybir
from concourse.bass_utils import run_bass_kernel_spmd

# Pin every ACTIVATE to the one table set that holds ln+exp+square, so the
# kernel pays a single ACT_TABLE_LOAD instead of ping-ponging between sets.
_PIN_SET = "natural_log_exp_and_others"
_orig_get_tables = hw_specs.get_activation_tables


def _pinned_tables(arch):
    tabs = _orig_get_tables(arch)
    return {k: (v if k == _PIN_SET else set()) for k, v in tabs.items()}


bacc.get_activation_tables = _pinned_tables

B, C, S = 2, 256, 512
P, K = 128, 512
BW = 0.05
EPS = 1e-5
N_CORES = 8
EOFF = 1.5  # exponent offset: E' = exp(cs/bw - EOFF); cancels in pos/sums
SF = 4.0  # fp8 scale on normalized features; ln(ibw/SF) folded into col bias
F32 = mybir.dt.float32
BF16 = mybir.dt.bfloat16
FP8 = mybir.dt.float8e4
DR = mybir.MatmulPerfMode.DoubleRow

GSQ = os.environ.get("K_GSQ") != "0"  # fy rest-squares on GpSimd
ASQ = os.environ.get("K_ASQ") != "0"  # fx rest-squares on ACT
TRE = os.environ.get("K_TRE") != "0"  # E^T(0,0) tile via PE transpose

# grp8 layout (bytes per partition row):
#   ones 0:256 | fyq 256:512 | fxq 512:768 | fxr 768:1536 | fyr 1536:2304
#   | cnt 2304:2816 | idh 2816:2944 | id8 2944:3072
G8W = 3072


def _build_program():
    nc = bacc.Bacc("TRN2", target_bir_lowering=False, debug=False, num_devices=N_CORES)

    grp8 = nc.dram_tensor("grp8", [128, G8W], FP8, kind="ExternalInput")
    partial = nc.dram_tensor("partial", [128, 4], F32, kind="ExternalOutput")

    with tile.TileContext(nc) as tc:
        with (
            tc.tile_pool(name="feat", bufs=1) as featp,
            tc.tile_pool(name="big", bufs=1) as big,
            tc.tile_pool(name="small", bufs=1) as small,
            tc.tile_pool(name="ps_row", bufs=1, space="PSUM") as ps_row,
            tc.tile_pool(name="ps_e", bufs=1, space="PSUM") as ps_e,
            tc.tile_pool(name="ps_et", bufs=1, space="PSUM") as ps_et,
            tc.tile_pool(name="ps_cr", bufs=1, space="PSUM") as ps_cr,
        ):
            hp = tc.high_priority

            # ---- input DMAs first: land while the engines boot; the q-block
            # features + fx rest ride the first DMA (they gate everything) ----
            t8 = featp.tile([128, G8W], FP8, name="t8", tag="t8")
            nc.sync.dma_start(out=t8[:, 0:1536], in_=grp8[:, 0:1536])
            nc.sync.dma_start(out=t8[:, 1536:2304], in_=grp8[:, 1536:2304])
            nc.sync.dma_start(out=t8[:, 2304:G8W], in_=grp8[:, 2304:G8W])

            # bias columns (ACT bias must be an AP)
            def bias_col(val, nm):
                t = small.tile([128, 1], F32, name=nm, tag=nm)
                nc.vector.memset(t, val)
                return t

            b_lnsf = bias_col(math.log(SF), "b_lnsf")
            b_lncol = bias_col(math.log(1.0 / BW / SF), "b_lncol")
            b_eoff = bias_col(-EOFF, "b_eoff")
            b_eps = bias_col(EPS, "b_eps")

            ones8w = t8[:, 0:256].rearrange("p (i s) -> p i s", i=2)
            ones_col = t8[:, 0:1]
            fyq = t8[:, 256:512].rearrange("p (i s) -> p i s", i=2)
            fxq = t8[:, 512:768].rearrange("p (i s) -> p i s", i=2)
            fxr = t8[:, 768:1536].rearrange("p (i s) -> p i s", i=2)
            fyr = t8[:, 1536:2304].rearrange("p (i s) -> p i s", i=2)
            cnt = [t8[:, 2304 + 128 * a : 2304 + 128 * (a + 1)] for a in range(4)]
            idh = t8[:, 2816:2944]
            id8 = t8[:, 2944:3072]

            def fxblk(a):
                return fxq if a == 0 else fxr[:, :, 128 * (a - 1) : 128 * a]

            def fyblk(a):
                return fyq if a == 0 else fyr[:, :, 128 * (a - 1) : 128 * a]

            # ---- q-block squares: ONE 4-D strided TT over fyq+fxq.
            # sqbuf free layout (i, side, s): [fy_i0 | fx_i0 | fy_i1 | fx_i1]
            # so the DR view (i, c) has c = (side, s) matching ssrow cols ----
            sqbuf = big.tile([128, 512], FP8, name="sqbuf", tag="sqbuf")
            sq4 = sqbuf.rearrange("p (i j s) -> p i j s", i=2, j=2)
            with hp():
                nc.vector.tensor_tensor(
                    out=sq4[:, :, 0], in0=fyq, in1=fyq, op=mybir.AluOpType.mult
                )
                nc.vector.tensor_tensor(
                    out=sq4[:, :, 1], in0=fxq, in1=fxq, op=mybir.AluOpType.mult
                )

            def sq_q(side, i):  # [128,128] view of the q-block squares
                return sqbuf[:, 256 * i + 128 * side : 256 * i + 128 * (side + 1)]

            # fx rest-squares on ACT: it is idle while the first DMA lands
            sq2r = big.tile([128, 2, 384], FP8, name="sq2r", tag="sq2r")
            if ASQ:
                nc.scalar.square(out=sq2r, in_=fxr)
            else:
                nc.vector.tensor_tensor(
                    out=sq2r, in0=fxr, in1=fxr, op=mybir.AluOpType.mult
                )

            # row-replicated channel sums of the q-block squares: one DR
            # matmul per side against an all-ones stationary, so the fy norm
            # chain starts without waiting for the fx squares
            ssrow_fy = ps_row.tile([128, 128], F32, name="ssrow_fy", tag="ssfy")
            ssfxcol = ps_row.tile([128, 136], F32, name="ssfxcol", tag="ssfx")
            ssrow_fx = ssfxcol[:, 0:128]
            with hp():
                nc.tensor.matmul(
                    out=ssrow_fy, lhsT=ones8w, rhs=sq4[:, :, 0],
                    perf_mode=DR, start=True, stop=True,
                )

            # fy rest-squares on the otherwise-idle GpSimd (gates only the
            # late E^T exps)
            sq1r = big.tile([128, 2, 384], FP8, name="sq1r", tag="sq1r")
            if GSQ:
                nc.gpsimd.tensor_tensor(
                    out=sq1r, in0=fyr, in1=fyr, op=mybir.AluOpType.mult
                )
            else:
                nc.vector.tensor_tensor(
                    out=sq1r, in0=fyr, in1=fyr, op=mybir.AluOpType.mult
                )

            # ---- moving-column normalization, fy side first (it gates every
            # E Gram); inv-norms come out row-replicated, no transposes ----
            lnrow = big.tile([128, 256], F32, name="lnrow", tag="lnrow")
            invrowq = big.tile([128, 256], BF16, name="invrowq", tag="invrowq")
            fyn = small.tile([128, 2, 128], FP8, name="fyn", tag="fyn")
            fxn = small.tile([128, 2, 128], FP8, name="fxn", tag="fxn")
            with hp():
                nc.scalar.activation(
                    out=lnrow[:, 0:128], in_=ssrow_fy,
                    func=mybir.ActivationFunctionType.Ln,
                )
                nc.scalar.activation(
                    out=invrowq[:, 0:128], in_=lnrow[:, 0:128],
                    func=mybir.ActivationFunctionType.Exp,
                    scale=-0.5, bias=b_lnsf,
                )
                for i in range(2):
                    nc.vector.tensor_tensor(
                        out=fyn[:, i, :], in0=fyq[:, i, :], in1=invrowq[:, 0:128],
                        op=mybir.AluOpType.mult,
                    )
            nc.tensor.matmul(
                out=ssrow_fx, lhsT=ones8w, rhs=sq4[:, :, 1],
                perf_mode=DR, start=True, stop=True,
            )
            nc.scalar.activation(
                out=lnrow[:, 128:256], in_=ssrow_fx,
                func=mybir.ActivationFunctionType.Ln,
            )
            nc.scalar.activation(
                out=invrowq[:, 128:256], in_=lnrow[:, 128:256],
                func=mybir.ActivationFunctionType.Exp,
                scale=-0.5, bias=b_lnsf,
            )
            for i in range(2):
                nc.vector.tensor_tensor(
                    out=fxn[:, i, :], in0=fxq[:, i, :], in1=invrowq[:, 128:256],
                    op=mybir.AluOpType.mult,
                )

            # ---- E col-block Grams (gated only on fyn) ----
            ebpair = [
                ps_e.tile([128, 256], F32, name=f"ebank{h}", tag=f"eb{h}")
                for h in range(2)
            ]
            ebank = [ebpair[a // 2][:, 128 * (a % 2) : 128 * (a % 2 + 1)] for a in range(4)]
            for a in range(4):
                nc.tensor.matmul(
                    out=ebank[a], lhsT=fxblk(a), rhs=fyn,
                    perf_mode=DR, start=True, stop=True,
                )

            # ---- per-partition exp scales: column sums of the squares ----
            sscol = ssfxcol[:, 128:136]

            def emit_colsums(side):  # side 0 = fx (cols 0:4), 1 = fy (4:8)
                sqr = sq2r if side == 0 else sq1r
                for a in range(4):
                    for i in range(2):
                        lhs = (
                            sq_q(side ^ 1, i)  # fy is side 0 in sqbuf
                            if a == 0
                            else sqr[:, i, 128 * (a - 1) : 128 * a]
                        )
                        nc.tensor.matmul(
                            out=sscol[:, 4 * side + a : 4 * side + a + 1],
                            lhsT=lhs, rhs=ones_col,
                            start=(i == 0), stop=(i == 1),
                        )

            def emit_invcol(side):
                lncol = small.tile([128, 4], F32, name=f"lncol{side}", tag=f"lnc{side}")
                invcol = small.tile([128, 4], F32, name=f"invcol{side}", tag=f"ivc{side}")
                nc.scalar.activation(
                    out=lncol, in_=sscol[:, 4 * side : 4 * side + 4],
                    func=mybir.ActivationFunctionType.Ln,
                )
                nc.scalar.activation(
                    out=invcol, in_=lncol,
                    func=mybir.ActivationFunctionType.Exp,
                    scale=-0.5, bias=b_lncol,
                )
                return invcol

            emit_colsums(0)
            inv2col = emit_invcol(0)

            # ---- E exps into per-block tiles (no shared-tile WAR with the
            # CS reads -> the exp stream runs back-to-back), CS accumulation ----
            e8 = [
                big.tile([128, 128], FP8, name=f"e8_{a}", tag=f"e8_{a}")
                for a in range(4)
            ]
            for a in range(4):
                nc.scalar.activation(
                    out=e8[a], in_=ebank[a],
                    func=mybir.ActivationFunctionType.Exp,
                    scale=inv2col[:, a : a + 1], bias=b_eoff,
                )
            cs_ps = ps_cr.tile([128, 128], F32, name="cs_ps", tag="cs_ps")
            for a in range(4):
                nc.tensor.matmul(
                    out=cs_ps, lhsT=e8[a], rhs=cnt[a],
                    start=(a == 0), stop=(a == 3),
                )

            cinv = small.tile([128, 128], F32, name="cinv", tag="cinv")
            nc.vector.reciprocal_approx_fast(out=cinv, in_=cs_ps)
            scr = small.tile([128, 128], BF16, name="scr", tag="scr")
            dcol = small.tile([128, 1], F32, name="dcol", tag="dcol")
            nc.vector.tensor_tensor(
                out=scr, in0=e8[0], in1=idh, op=mybir.AluOpType.mult
            )
            nc.vector.tensor_reduce(
                out=dcol, in_=scr, axis=mybir.AxisListType.X, op=mybir.AluOpType.add
            )

            # ---- E^T col-blocks: tile 0 is E(0,0) transposed; 1..3 by Gram ----
            et8 = [
                big.tile([128, 128], FP8, name=f"et8_{a}", tag=f"et8_{a}")
                for a in range(4)
            ]
            etb = ps_et.tile([128, 384], F32, name="etbank", tag="etbank")
            etbank = [etb[:, 128 * h : 128 * (h + 1)] for h in range(3)]
            a0 = 1 if TRE else 0
            if TRE:
                # fp8 transpose writes PSUM with element step 2: stage in a
                # [128, 128, 2] tile and use the stride-2 view as the output
                et_full = ps_et.tile([128, 128, 2], FP8, name="et_ps", tag="et_ps")
                nc.tensor.transpose(
                    out=et_full[:, :, 0], in_=e8[0], identity=id8
                )
                nc.vector.tensor_copy(out=et8[0], in_=et_full[:, :, 0])
            for a in range(a0, 4):
                nc.tensor.matmul(
                    out=etbank[a - 1], lhsT=fyblk(a), rhs=fxn,
                    perf_mode=DR, start=True, stop=True,
                )

            emit_colsums(1)
            inv1col = emit_invcol(1)

            for a in range(a0, 4):
                nc.scalar.activation(
                    out=et8[a], in_=etbank[a - 1],
                    func=mybir.ActivationFunctionType.Exp,
                    scale=inv1col[:, a : a + 1], bias=b_eoff,
                )
            rs_ps = ps_cr.tile([128, 128], F32, name="rs_ps", tag="rs_ps")
            for a in range(4):
                nc.tensor.matmul(
                    out=rs_ps, lhsT=et8[a], rhs=cnt[a],
                    start=(a == 0), stop=(a == 3),
                )

            # ---- tail: sum_c c * ln(0.5*d*(1/RS+1/CS) + eps) ----
            # RS/CS are sums of positive e-values; ~18-bit recip is plenty
            rinv = small.tile([128, 128], F32, name="rinv", tag="rinv")
            nc.vector.reciprocal_approx_fast(out=rinv, in_=rs_ps)
            ssum = small.tile([128, 128], F32, name="ssum", tag="ssum")
            nc.vector.tensor_tensor(
                out=ssum, in0=rinv, in1=cinv, op=mybir.AluOpType.add
            )
            gl = small.tile([128, 128], F32, name="gl", tag="gl")
            nc.scalar.activation(
                out=gl, in_=ssum, func=mybir.ActivationFunctionType.Ln,
                scale=dcol, bias=b_eps,
            )
            # acc kept at 4 columns: a [128,1] f32 output makes 4-byte DMA
            # descriptors, which lands the NEFF in a much slower teardown path
            acc = small.tile([128, 4], F32, name="acc", tag="acc")
            nc.vector.memset(acc[:, 1:4], 0.0)
            wgl = small.tile([128, 128], F32, name="wgl", tag="wgl")
            nc.vector.tensor_tensor(
                out=wgl, in0=gl, in1=cnt[0], op=mybir.AluOpType.mult
            )
            nc.vector.tensor_reduce(
                out=acc[:, 0:1], in_=wgl, axis=mybir.AxisListType.X,
                op=mybir.AluOpType.add,
            )
            nc.sync.dma_start(out=partial[:, :], in_=acc)

    nc.compile()
    return nc


_NC = None


def _pack_inputs(t2, t1, idx):
    counts = np.zeros((P, S), np.float32)
    np.add.at(counts, (np.arange(P)[:, None], idx), 1.0)
    countsT = counts.T  # (S, P)
    idh = 0.5 * np.eye(128, dtype=np.float32)
    id8 = np.eye(128, dtype=np.float32)
    ones = np.ones((128, 256), np.float32)

    in_maps = []
    for core in range(N_CORES):
        b, q = divmod(core, 4)
        qblk = np.arange(128 * q, 128 * (q + 1))
        perm = np.concatenate([qblk, np.delete(np.arange(S), qblk)])
        # features with channel dim split (i, p): c = 128*i + p
        fx = t2[b][:, perm].reshape(2, 128, S).transpose(1, 0, 2)  # [128,2,S]
        fy = t1[b][:, perm].reshape(2, 128, S).transpose(1, 0, 2)
        cntp = countsT[perm].reshape(4, 128, 128).transpose(1, 0, 2).reshape(128, 512)
        grp8 = np.concatenate(
            [
                ones,
                fy[:, :, 0:128].reshape(128, 256),
                fx[:, :, 0:128].reshape(128, 256),
                fx[:, :, 128:512].reshape(128, 768),
                fy[:, :, 128:512].reshape(128, 768),
                cntp,
                idh,
                id8,
            ],
            axis=1,
        ).astype(ml_dtypes.float8_e4m3fn)
        in_maps.append({"grp8": grp8})
    return in_maps


def _run(t2_feat, t1_feat, idx, trace=False, trace_kwargs=None):
    global _NC
    if _NC is None:
        _NC = _build_program()

    t2 = np.ascontiguousarray(np.asarray(t2_feat, np.float32).reshape(B, C, S))
    t1 = np.ascontiguousarray(np.asarray(t1_feat, np.float32).reshape(B, C, S))
    idx = np.asarray(idx)
    in_maps = _pack_inputs(t2, t1, idx)

    kwargs = {}
    if trace:
        kwargs = dict(trace=True, trace_kwargs=trace_kwargs or {})
    res = run_bass_kernel_spmd(_NC, in_maps, core_ids=list(range(N_CORES)), **kwargs)
    total = sum(r["partial"].sum(dtype=np.float64) for r in res.results)
    loss = -total / (P * B * K)
    return np.array(loss, dtype=np.float32), res


def kernel(t2_feat, t1_feat, idx):
    out, _ = _run(t2_feat, t1_feat, idx)
    return out
